# revision 33
# baseline (speedup 1.0000x reference)
"""Trainium2 Bass kernel for DrafterAttention (decode attention, B=8 H=16 D=128 S=4096 HID=2048).

Strategy (tensor-parallel over heads, 8 cores x 2 heads):
  - Host: shard Wq columns / Wo rows / kv on the head axis; pre-transpose
    kv_k -> (B,HC,D,S) and pre-tile kv_v -> (B,HC,128,NCH*128), packed into one
    kv tensor, so every device-side DMA moves contiguous per-partition lines
    and every matmul consumes natural SBUF layouts (x is transposed on-device
    via a PE transpose).
  - Device (per core): qT = Wq_shard^T @ x^T on the PE; RMS-norm + RoPE in a
    (d-on-partition, batch-on-free) layout; per (b,h): 32 matmuls
    kT_chunk^T @ q_col -> scores (128s x 32chunk) in one PSUM accumulation
    group; exp via ACT with fused row-sum (no max subtraction: logits are O(1)
    by construction); partition-sum broadcast via ones-matmul; 32 accumulating
    matmuls vT_chunk^T @ prob_col -> attention head column; o_proj partial
    (8 x 2048) per core.
  - Host: sum the 8 partial outputs (the all-reduce).

All matmuls run in KV_DTYPE ("bf16" halves HBM traffic and runs the PE weight
path at FWL speed; "f32" is the exact-but-slower fallback). Accumulation is
always fp32 in PSUM; softmax statistics are kept in fp32.
"""
import numpy as np

B, H, D, S, HID = 8, 16, 128, 4096, 2048
NCORES = 8
HC = H // NCORES          # 2 heads per core
NCH = S // 128            # 32 s-chunks
SCALE = 1.0 / np.sqrt(D)
EPS = 1e-6

KV_DTYPE = "bf16"         # "bf16" | "f32"

_CACHE = {}


def _split_sync_waits(nc, max_waits=1):
    """This walrus build caps per-instruction sem waits; hoist any excess
    onto NoOp instructions inserted just before, on the same engine."""
    from concourse import mybir
    import bass_rust

    n = 0
    for fn in nc.m.functions:
        for blk in fn.blocks:
            new_list = []
            changed = False
            for inst in blk.instructions:
                si = inst.sync_info
                waits = list(si.on_wait) if (si and si.on_wait) else []
                if len(waits) > max_waits:
                    extra, keep = waits[:-max_waits], waits[-max_waits:]
                    for i in range(0, len(extra), max_waits):
                        n += 1
                        nop = bass_rust.InstNoOp(
                            name=f"I-waitsplit-{n}", ins=[], outs=[])
                        nop.engine = inst.engine
                        nop.sync_info = mybir.SyncInfo(
                            on_wait=extra[i:i + max_waits], on_update=[])
                        new_list.append(nop)
                    si.on_wait = keep
                    changed = True
                new_list.append(inst)
            if changed:
                blk.instructions[:] = new_list
    return n


def _build_nc(kv_dtype=KV_DTYPE):
    from contextlib import ExitStack
    import concourse.bass as bass
    import concourse.tile as tile
    from concourse import mybir

    f32 = mybir.dt.float32
    kv_dt = mybir.dt.bfloat16 if kv_dtype == "bf16" else f32

    nc = bass.Bass(trn_type="TRN2")

    # xTp[p, i*B+b] = x[b, i*128+p] — host-packed transposed x
    xTp = nc.dram_tensor("xTp", [128, (HID // 128) * B], kv_dt,
                         kind="ExternalInput")
    wq = nc.dram_tensor("wq", [HID, HC * D], kv_dt, kind="ExternalInput")
    wo = nc.dram_tensor("wo", [HC * D, HID], kv_dt, kind="ExternalInput")
    # kv packed per (b,h): [kT (128,S) | vT pre-tiled (128,S)], 16KB lines
    kv = nc.dram_tensor("kv", [B, HC, 128, 2 * S], kv_dt, kind="ExternalInput")
    # smalls: col0 = [cos;sin], col1 = gamma*SCALE, cols 2.. = mask tiles
    # (sm[p, 2 + b*NCH + c] = mask[b, c*128 + p])
    sm = nc.dram_tensor("sm", [128, 2 + B * NCH], f32, kind="ExternalInput")
    out = nc.dram_tensor("out", [B, HID], f32, kind="ExternalOutput")

    with ExitStack() as ctx:
        tc = ctx.enter_context(tile.TileContext(nc))

        consts = ctx.enter_context(tc.tile_pool(name="consts", bufs=1))
        qpool = ctx.enter_context(tc.tile_pool(name="qpool", bufs=1))
        kvpool = ctx.enter_context(
            tc.tile_pool(name="kvpool", bufs=6 if kv_dtype == "bf16" else 3))
        spool = ctx.enter_context(tc.tile_pool(name="spool", bufs=3))

        # ---- constants / small inputs ----
        ones_sb = consts.tile([128, 128], f32)
        nc.gpsimd.memset(ones_sb[:], 1.0)
        eps_sb = consts.tile([128, 1], f32)
        nc.gpsimd.memset(eps_sb[:], EPS)
        sm_sb = consts.tile([128, 2 + B * NCH], f32)
        nc.gpsimd.dma_start(sm_sb[:], sm[:])
        csg_sb = sm_sb[:, 0:2]
        wq_sb = consts.tile([128, HID // 128, HC * D], kv_dt)
        wq_r = wq[:].rearrange("(i p) j -> p i j", p=128)
        for piece in range(4):
            nc.scalar.dma_start(wq_sb[:, piece * 4:(piece + 1) * 4, :],
                                wq_r[:, piece * 4:(piece + 1) * 4, :])
        wo_sb = consts.tile([128, HC, HID], kv_dt)
        xT_sb = consts.tile([128, HID // 128, B], kv_dt)
        nc.gpsimd.dma_start(
            xT_sb[:], xTp[:].rearrange("p (i b) -> p i b", b=B))
        mask_tiles = [sm_sb[:, 2 + b * NCH:2 + (b + 1) * NCH] for b in range(B)]

        # ---- q projection: qT_h = (Wq_h)^T @ x^T  -> (128d, B) per head ----
        q_heads = []
        with tc.tile_pool(name="psQ", bufs=1, space="PSUM") as psq:
            for h in range(HC):
                q_ps = psq.tile([128, B], f32, name="qps")
                for i in range(HID // 128):
                    nc.tensor.matmul(
                        q_ps[:],
                        wq_sb[:, i, h * D:(h + 1) * D],
                        xT_sb[:, i, :],
                        start=(i == 0), stop=(i == HID // 128 - 1),
                    )
                # RMS norm (over the partition axis d) via ones-matmul
                qs = qpool.tile([128, 3 * B], f32, name=f"qs{h}")
                sq = qs[:, 0:B]
                rms = qs[:, B:2 * B]
                qn = qs[:, 2 * B:3 * B]
                nc.scalar.square(sq, q_ps[:])
                ssq_ps = psq.tile([128, B], f32, name="ssq")
                nc.tensor.matmul(ssq_ps[:], ones_sb[:], sq, start=True, stop=True)
                nc.scalar.activation(rms, ssq_ps[:],
                                     mybir.ActivationFunctionType.Sqrt,
                                     bias=eps_sb[:], scale=1.0 / D)
                nc.vector.reciprocal(rms, rms)
                nc.vector.tensor_mul(qn, q_ps[:], rms)
                # gamma * SCALE (per-partition scalar)
                nc.vector.tensor_scalar_mul(qn, qn, sm_sb[:, 1:2])
                # RoPE on partition halves: cos/sin stacked in csg col 0;
                # t1/t2 reuse the dead sq/rms columns
                qr = qpool.tile([128, B], kv_dt, name=f"qr{h}")
                t1 = qs[0:64, 0:B]
                t2 = qs[0:64, B:2 * B]
                cos_ap = sm_sb[0:64, 0:1]
                sin_ap = sm_sb[64:128, 0:1]
                q1 = qn[0:64, :]
                q2 = qn[64:128, :]
                nc.vector.tensor_scalar_mul(t1, q1, cos_ap)
                nc.vector.tensor_scalar_mul(t2, q2, sin_ap)
                nc.vector.tensor_sub(qr[0:64, :], t1, t2)
                nc.vector.tensor_scalar_mul(t1, q2, cos_ap)
                nc.vector.tensor_scalar_mul(t2, q1, sin_ap)
                nc.vector.tensor_add(qr[64:128, :], t1, t2)
                q_heads.append(qr)

        # attention output columns, (128d, B) per head
        at_tiles = [qpool.tile([128, B], kv_dt, name=f"at{h}") for h in range(HC)]
        o_sb = qpool.tile([B, HID], f32, name="osb")

        # ---- main streamed attention loop (h-major; AV pipelined 1 unit
        # behind scores so the PE never waits on the ACT/DVE softmax chain) --
        units = [(h, b) for h in range(HC) for b in range(B)]
        # the LAST unit's kv is prefetched early (at u==4) so the tail
        # isn't DMA-bound
        lastpool = ctx.enter_context(tc.tile_pool(name="lastkv", bufs=1))
        hl, bl = units[-1]
        kv_last = lastpool.tile([128, 2 * S], kv_dt, name="kvlast")

        ps_sc = ctx.enter_context(tc.tile_pool(name="psS", bufs=2, space="PSUM"))
        ps_av = ctx.enter_context(tc.tile_pool(name="psV", bufs=2, space="PSUM"))
        ps_tot = ctx.enter_context(tc.tile_pool(name="psT", bufs=2, space="PSUM"))
        ps_o = ctx.enter_context(tc.tile_pool(name="psO", bufs=2, space="PSUM"))

        def emit_av(pend):
            kv_p, probs_p, swork_p, h_p, b_p = pend
            av_ps = ps_av.tile([128, 1], f32, name="avps")
            for c in range(NCH):
                nc.tensor.matmul(
                    av_ps[:],
                    kv_p[:, S + c * 128:S + (c + 1) * 128],
                    probs_p[:, c:c + 1],
                    start=(c == 0), stop=(c == NCH - 1),
                )
            tot_ps = ps_tot.tile([128, 1], f32, name="totps")
            nc.tensor.matmul(tot_ps[:], ones_sb[:],
                             swork_p[:, NCH:NCH + 1], start=True, stop=True)
            inv = swork_p[:, NCH + 1:NCH + 2]
            nc.vector.reciprocal(inv, tot_ps[:])
            nc.scalar.activation(at_tiles[h_p][:, b_p:b_p + 1], av_ps[:],
                                 mybir.ActivationFunctionType.Copy,
                                 scale=inv)

        def emit_oproj(h, first):
            # partial o_proj for one head into o_sb; final head streams out
            for n in range(HID // 512):
                o_ps = ps_o.tile([B, 512], f32, name="ops")
                nc.tensor.matmul(o_ps[:], at_tiles[h][:],
                                 wo_sb[:, h, n * 512:(n + 1) * 512],
                                 start=True, stop=True)
                if first:
                    nc.vector.tensor_copy(o_sb[:, n * 512:(n + 1) * 512], o_ps[:])
                else:
                    nc.vector.tensor_add(o_sb[:, n * 512:(n + 1) * 512],
                                         o_sb[:, n * 512:(n + 1) * 512], o_ps[:])
                    nc.sync.dma_start(out[:, n * 512:(n + 1) * 512],
                                      o_sb[:, n * 512:(n + 1) * 512])

        pending = None
        rings = [nc.sync, nc.scalar]
        for u, (h, b) in enumerate(units):
            last = (u == len(units) - 1)
            if u == 4:
                nc.scalar.dma_start(kv_last[:, 0:S], kv[bl, hl][:, 0:S])
                nc.scalar.dma_start(kv_last[:, S:2 * S], kv[bl, hl][:, S:2 * S])
            if last:
                kv_sb = kv_last
            else:
                kv_sb = kvpool.tile([128, 2 * S], kv_dt, name="kvtile")
                r = rings[u % 2]
                if 2 <= u < len(units) - 4:
                    # one big transfer: better sustained queue rate
                    r.dma_start(kv_sb[:], kv[b, h])
                else:
                    # tail units: k-half first so scores can start sooner
                    r.dma_start(kv_sb[:, 0:S], kv[b, h][:, 0:S])
                    r.dma_start(kv_sb[:, S:2 * S], kv[b, h][:, S:2 * S])
            if u == 2:
                nc.gpsimd.dma_start(
                    wo_sb[:], wo[:].rearrange("(h p) n -> p h n", p=128))

            q_col = q_heads[h][:, b:b + 1]
            sc_ps = ps_sc.tile([128, NCH], f32, name="scps")
            for c in range(NCH):
                nc.tensor.matmul(
                    sc_ps[:, c:c + 1],
                    kv_sb[:, c * 128:(c + 1) * 128],
                    q_col,
                    start=(c == 0), stop=(c == NCH - 1),
                )
            if pending is not None:
                emit_av(pending)
            swork = spool.tile([128, NCH + 2], f32, name="swork")
            sc_sb = swork[:, 0:NCH]
            rowsum = swork[:, NCH:NCH + 1]
            nc.vector.tensor_add(sc_sb, sc_ps[:], mask_tiles[b])
            probs = spool.tile([128, NCH], kv_dt, name="probs")
            nc.scalar.activation(probs[:], sc_sb,
                                 mybir.ActivationFunctionType.Exp,
                                 accum_out=rowsum)
            pending = (kv_sb, probs, swork, h, b)
            if u == B + 1:
                # head 0's attention columns are complete; project mid-loop
                emit_oproj(0, first=True)
        emit_av(pending)
        emit_oproj(1, first=False)

    _split_sync_waits(nc)
    return nc


def _get_nc():
    if "nc" not in _CACHE:
        _CACHE["nc"] = _build_nc()
    return _CACHE["nc"]


def _shard_inputs(x, kv_k, kv_v, cos, sin, mask, Wq, Wo, q_gamma, kv_dtype=KV_DTYPE):
    if kv_dtype == "bf16":
        import ml_dtypes
        kv_np = ml_dtypes.bfloat16
    else:
        kv_np = np.float32

    x = np.asarray(x, np.float32).reshape(B, HID)
    # xTp[p, i*B+b] = x[b, i*128+p]
    xtp = np.ascontiguousarray(
        x.reshape(B, HID // 128, 128).transpose(2, 1, 0)
        .reshape(128, (HID // 128) * B).astype(kv_np))
    sm = np.empty((128, 2 + B * NCH), np.float32)
    sm[:64, 0] = np.asarray(cos, np.float32).reshape(-1)
    sm[64:, 0] = np.asarray(sin, np.float32).reshape(-1)
    sm[:, 1] = np.asarray(q_gamma, np.float32).reshape(-1) * SCALE
    sm[:, 2:] = (np.asarray(mask, np.float32).reshape(B, NCH, 128)
                 .transpose(2, 0, 1).reshape(128, B * NCH))
    kv_k = np.asarray(kv_k, np.float32)
    kv_v = np.asarray(kv_v, np.float32)
    Wq = np.asarray(Wq, np.float32)
    Wo = np.asarray(Wo, np.float32)

    in_maps = []
    for c in range(NCORES):
        hs = c * HC
        # packed per (b,h): [kT (128,S) | vT pre-tiled (128,S)]
        kvp = np.empty((B, HC, 128, 2 * S), kv_np)
        kvp[..., :S] = kv_k[:, hs:hs + HC].transpose(0, 1, 3, 2)
        kvp[..., S:] = (kv_v[:, hs:hs + HC].reshape(B, HC, D, NCH, 128)
                        .transpose(0, 1, 4, 3, 2).reshape(B, HC, 128, S))
        in_maps.append({
            "xTp": xtp,
            "wq": np.ascontiguousarray(Wq[:, hs * D:(hs + HC) * D].astype(kv_np)),
            "wo": np.ascontiguousarray(Wo[hs * D:(hs + HC) * D, :].astype(kv_np)),
            "kv": kvp,
            "sm": sm,
        })
    return in_maps


def kernel(x, kv_k, kv_v, cos, sin, mask, Wq, Wo, q_gamma, _trace=False):
    from concourse.bass_utils import run_bass_kernel_spmd

    nc = _get_nc()
    in_maps = _shard_inputs(x, kv_k, kv_v, cos, sin, mask, Wq, Wo, q_gamma)
    res = run_bass_kernel_spmd(nc, in_maps, list(range(NCORES)), trace=_trace)
    acc = np.zeros((B, HID), np.float64)
    for c in range(NCORES):
        acc += res.results[c]["out"].astype(np.float64)
    out = acc.astype(np.float32).reshape(B, 1, HID)
    if _trace:
        return out, res
    return out


# revision 34
# speedup vs baseline: 1.0043x; 1.0043x over previous
"""Trainium2 Bass kernel for DrafterAttention (decode attention, B=8 H=16 D=128 S=4096 HID=2048).

Strategy (tensor-parallel over heads, 8 cores x 2 heads):
  - Host: shard Wq columns / Wo rows / kv on the head axis; pre-transpose
    kv_k -> (B,HC,D,S) and pre-tile kv_v -> (B,HC,128,NCH*128), packed into one
    kv tensor, so every device-side DMA moves contiguous per-partition lines
    and every matmul consumes natural SBUF layouts (x is transposed on-device
    via a PE transpose).
  - Device (per core): qT = Wq_shard^T @ x^T on the PE; RMS-norm + RoPE in a
    (d-on-partition, batch-on-free) layout; per (b,h): 32 matmuls
    kT_chunk^T @ q_col -> scores (128s x 32chunk) in one PSUM accumulation
    group; exp via ACT with fused row-sum (no max subtraction: logits are O(1)
    by construction); partition-sum broadcast via ones-matmul; 32 accumulating
    matmuls vT_chunk^T @ prob_col -> attention head column; o_proj partial
    (8 x 2048) per core.
  - Host: sum the 8 partial outputs (the all-reduce).

All matmuls run in KV_DTYPE ("bf16" halves HBM traffic and runs the PE weight
path at FWL speed; "f32" is the exact-but-slower fallback). Accumulation is
always fp32 in PSUM; softmax statistics are kept in fp32.
"""
import numpy as np

B, H, D, S, HID = 8, 16, 128, 4096, 2048
NCORES = 8
HC = H // NCORES          # 2 heads per core
NCH = S // 128            # 32 s-chunks
SCALE = 1.0 / np.sqrt(D)
EPS = 1e-6

KV_DTYPE = "bf16"         # "bf16" | "f32"

_CACHE = {}


def _split_sync_waits(nc, max_waits=1):
    """This walrus build caps per-instruction sem waits; hoist any excess
    onto NoOp instructions inserted just before, on the same engine."""
    from concourse import mybir
    import bass_rust

    n = 0
    for fn in nc.m.functions:
        for blk in fn.blocks:
            new_list = []
            changed = False
            for inst in blk.instructions:
                si = inst.sync_info
                waits = list(si.on_wait) if (si and si.on_wait) else []
                if len(waits) > max_waits:
                    extra, keep = waits[:-max_waits], waits[-max_waits:]
                    for i in range(0, len(extra), max_waits):
                        n += 1
                        nop = bass_rust.InstNoOp(
                            name=f"I-waitsplit-{n}", ins=[], outs=[])
                        nop.engine = inst.engine
                        nop.sync_info = mybir.SyncInfo(
                            on_wait=extra[i:i + max_waits], on_update=[])
                        new_list.append(nop)
                    si.on_wait = keep
                    changed = True
                new_list.append(inst)
            if changed:
                blk.instructions[:] = new_list
    return n


def _build_nc(kv_dtype=KV_DTYPE):
    from contextlib import ExitStack
    import concourse.bass as bass
    import concourse.tile as tile
    from concourse import mybir

    f32 = mybir.dt.float32
    kv_dt = mybir.dt.bfloat16 if kv_dtype == "bf16" else f32

    nc = bass.Bass(trn_type="TRN2")

    # xTp[p, i*B+b] = x[b, i*128+p] — host-packed transposed x
    xTp = nc.dram_tensor("xTp", [128, (HID // 128) * B], kv_dt,
                         kind="ExternalInput")
    wq = nc.dram_tensor("wq", [HID, HC * D], kv_dt, kind="ExternalInput")
    wo = nc.dram_tensor("wo", [HC * D, HID], kv_dt, kind="ExternalInput")
    # kv packed per (b,h): [kT (128,S) | vT pre-tiled (128,S)], 16KB lines
    kv = nc.dram_tensor("kv", [B, HC, 128, 2 * S], kv_dt, kind="ExternalInput")
    # smalls: col0 = [cos;sin], col1 = gamma*SCALE, cols 2.. = mask tiles
    # (sm[p, 2 + b*NCH + c] = mask[b, c*128 + p])
    sm = nc.dram_tensor("sm", [128, 2 + B * NCH], f32, kind="ExternalInput")
    out = nc.dram_tensor("out", [B, HID], f32, kind="ExternalOutput")

    with ExitStack() as ctx:
        tc = ctx.enter_context(tile.TileContext(nc))

        consts = ctx.enter_context(tc.tile_pool(name="consts", bufs=1))
        qpool = ctx.enter_context(tc.tile_pool(name="qpool", bufs=1))
        kvpool = ctx.enter_context(
            tc.tile_pool(name="kvpool", bufs=6 if kv_dtype == "bf16" else 3))
        spool = ctx.enter_context(tc.tile_pool(name="spool", bufs=3))

        # ---- constants / small inputs ----
        ones_sb = consts.tile([128, 128], f32)
        nc.gpsimd.memset(ones_sb[:], 1.0)
        eps_sb = consts.tile([128, 1], f32)
        nc.gpsimd.memset(eps_sb[:], EPS)
        sm_sb = consts.tile([128, 2 + B * NCH], f32)
        nc.gpsimd.dma_start(sm_sb[:], sm[:])
        csg_sb = sm_sb[:, 0:2]
        wq_sb = consts.tile([128, HID // 128, HC * D], kv_dt)
        wq_r = wq[:].rearrange("(i p) j -> p i j", p=128)
        for piece in range(4):
            nc.scalar.dma_start(wq_sb[:, piece * 4:(piece + 1) * 4, :],
                                wq_r[:, piece * 4:(piece + 1) * 4, :])
        wo_sb = consts.tile([128, HC, HID], kv_dt)
        xT_sb = consts.tile([128, HID // 128, B], kv_dt)
        nc.gpsimd.dma_start(
            xT_sb[:], xTp[:].rearrange("p (i b) -> p i b", b=B))
        mask_tiles = [sm_sb[:, 2 + b * NCH:2 + (b + 1) * NCH] for b in range(B)]

        # ---- q projection: qT_h = (Wq_h)^T @ x^T  -> (128d, B) per head ----
        q_heads = []
        with tc.tile_pool(name="psQ", bufs=1, space="PSUM") as psq:
            for h in range(HC):
                q_ps = psq.tile([128, B], f32, name="qps")
                for i in range(HID // 128):
                    nc.tensor.matmul(
                        q_ps[:],
                        wq_sb[:, i, h * D:(h + 1) * D],
                        xT_sb[:, i, :],
                        start=(i == 0), stop=(i == HID // 128 - 1),
                    )
                # RMS norm (over the partition axis d) via ones-matmul
                qs = qpool.tile([128, 3 * B], f32, name=f"qs{h}")
                sq = qs[:, 0:B]
                rms = qs[:, B:2 * B]
                qn = qs[:, 2 * B:3 * B]
                nc.scalar.square(sq, q_ps[:])
                ssq_ps = psq.tile([128, B], f32, name="ssq")
                nc.tensor.matmul(ssq_ps[:], ones_sb[:], sq, start=True, stop=True)
                nc.scalar.activation(rms, ssq_ps[:],
                                     mybir.ActivationFunctionType.Sqrt,
                                     bias=eps_sb[:], scale=1.0 / D)
                nc.vector.reciprocal(rms, rms)
                nc.vector.tensor_mul(qn, q_ps[:], rms)
                # gamma * SCALE (per-partition scalar)
                nc.vector.tensor_scalar_mul(qn, qn, sm_sb[:, 1:2])
                # RoPE on partition halves: cos/sin stacked in csg col 0;
                # t1/t2 reuse the dead sq/rms columns
                qr = qpool.tile([128, B], kv_dt, name=f"qr{h}")
                t1 = qs[0:64, 0:B]
                t2 = qs[0:64, B:2 * B]
                cos_ap = sm_sb[0:64, 0:1]
                sin_ap = sm_sb[64:128, 0:1]
                q1 = qn[0:64, :]
                q2 = qn[64:128, :]
                nc.vector.tensor_scalar_mul(t1, q1, cos_ap)
                nc.vector.tensor_scalar_mul(t2, q2, sin_ap)
                nc.vector.tensor_sub(qr[0:64, :], t1, t2)
                nc.vector.tensor_scalar_mul(t1, q2, cos_ap)
                nc.vector.tensor_scalar_mul(t2, q1, sin_ap)
                nc.vector.tensor_add(qr[64:128, :], t1, t2)
                q_heads.append(qr)

        # attention output columns, (128d, B) per head
        at_tiles = [qpool.tile([128, B], kv_dt, name=f"at{h}") for h in range(HC)]
        o_sb = qpool.tile([B, HID], f32, name="osb")

        # ---- main streamed attention loop (h-major; AV pipelined 1 unit
        # behind scores so the PE never waits on the ACT/DVE softmax chain) --
        units = [(h, b) for h in range(HC) for b in range(B)]
        # the LAST unit's kv is prefetched early (at u==4) so the tail
        # isn't DMA-bound
        lastpool = ctx.enter_context(tc.tile_pool(name="lastkv", bufs=1))
        hl, bl = units[-1]
        kv_last = lastpool.tile([128, 2 * S], kv_dt, name="kvlast")

        ps_sc = ctx.enter_context(tc.tile_pool(name="psS", bufs=2, space="PSUM"))
        ps_av = ctx.enter_context(tc.tile_pool(name="psV", bufs=2, space="PSUM"))
        ps_tot = ctx.enter_context(tc.tile_pool(name="psT", bufs=2, space="PSUM"))
        ps_o = ctx.enter_context(tc.tile_pool(name="psO", bufs=2, space="PSUM"))

        def emit_av(pend):
            kv_p, probs_p, swork_p, h_p, b_p = pend
            av_ps = ps_av.tile([128, 1], f32, name="avps")
            for c in range(NCH):
                nc.tensor.matmul(
                    av_ps[:],
                    kv_p[:, S + c * 128:S + (c + 1) * 128],
                    probs_p[:, c:c + 1],
                    start=(c == 0), stop=(c == NCH - 1),
                )
            tot_ps = ps_tot.tile([128, 1], f32, name="totps")
            nc.tensor.matmul(tot_ps[:], ones_sb[:],
                             swork_p[:, NCH:NCH + 1], start=True, stop=True)
            inv = swork_p[:, NCH + 1:NCH + 2]
            nc.vector.reciprocal(inv, tot_ps[:])
            nc.scalar.activation(at_tiles[h_p][:, b_p:b_p + 1], av_ps[:],
                                 mybir.ActivationFunctionType.Copy,
                                 scale=inv)

        def emit_oproj(h, first):
            # partial o_proj for one head into o_sb; final head streams out
            for n in range(HID // 512):
                o_ps = ps_o.tile([B, 512], f32, name="ops")
                nc.tensor.matmul(o_ps[:], at_tiles[h][:],
                                 wo_sb[:, h, n * 512:(n + 1) * 512],
                                 start=True, stop=True)
                if first:
                    nc.vector.tensor_copy(o_sb[:, n * 512:(n + 1) * 512], o_ps[:])
                else:
                    nc.vector.tensor_add(o_sb[:, n * 512:(n + 1) * 512],
                                         o_sb[:, n * 512:(n + 1) * 512], o_ps[:])
                    nc.scalar.dma_start(out[:, n * 512:(n + 1) * 512],
                                        o_sb[:, n * 512:(n + 1) * 512])

        pending = None
        rings = [nc.sync, nc.scalar]
        for u, (h, b) in enumerate(units):
            last = (u == len(units) - 1)
            if u == 4:
                nc.scalar.dma_start(kv_last[:, 0:S], kv[bl, hl][:, 0:S])
                nc.scalar.dma_start(kv_last[:, S:2 * S], kv[bl, hl][:, S:2 * S])
            if last:
                kv_sb = kv_last
            else:
                kv_sb = kvpool.tile([128, 2 * S], kv_dt, name="kvtile")
                r = rings[u % 2]
                if 2 <= u < len(units) - 4:
                    # one big transfer: better sustained queue rate
                    r.dma_start(kv_sb[:], kv[b, h])
                else:
                    # tail units: k-half first so scores can start sooner
                    r.dma_start(kv_sb[:, 0:S], kv[b, h][:, 0:S])
                    r.dma_start(kv_sb[:, S:2 * S], kv[b, h][:, S:2 * S])
            if u == 2:
                nc.gpsimd.dma_start(
                    wo_sb[:], wo[:].rearrange("(h p) n -> p h n", p=128))

            q_col = q_heads[h][:, b:b + 1]
            sc_ps = ps_sc.tile([128, NCH], f32, name="scps")
            for c in range(NCH):
                nc.tensor.matmul(
                    sc_ps[:, c:c + 1],
                    kv_sb[:, c * 128:(c + 1) * 128],
                    q_col,
                    start=(c == 0), stop=(c == NCH - 1),
                )
            if pending is not None:
                emit_av(pending)
            swork = spool.tile([128, NCH + 2], f32, name="swork")
            sc_sb = swork[:, 0:NCH]
            rowsum = swork[:, NCH:NCH + 1]
            nc.vector.tensor_add(sc_sb, sc_ps[:], mask_tiles[b])
            probs = spool.tile([128, NCH], kv_dt, name="probs")
            nc.scalar.activation(probs[:], sc_sb,
                                 mybir.ActivationFunctionType.Exp,
                                 accum_out=rowsum)
            pending = (kv_sb, probs, swork, h, b)
            if u == B + 1:
                # head 0's attention columns are complete; project mid-loop
                emit_oproj(0, first=True)
        emit_av(pending)
        emit_oproj(1, first=False)

    _split_sync_waits(nc)
    return nc


def _get_nc():
    if "nc" not in _CACHE:
        _CACHE["nc"] = _build_nc()
    return _CACHE["nc"]


def _shard_inputs(x, kv_k, kv_v, cos, sin, mask, Wq, Wo, q_gamma, kv_dtype=KV_DTYPE):
    if kv_dtype == "bf16":
        import ml_dtypes
        kv_np = ml_dtypes.bfloat16
    else:
        kv_np = np.float32

    x = np.asarray(x, np.float32).reshape(B, HID)
    # xTp[p, i*B+b] = x[b, i*128+p]
    xtp = np.ascontiguousarray(
        x.reshape(B, HID // 128, 128).transpose(2, 1, 0)
        .reshape(128, (HID // 128) * B).astype(kv_np))
    sm = np.empty((128, 2 + B * NCH), np.float32)
    sm[:64, 0] = np.asarray(cos, np.float32).reshape(-1)
    sm[64:, 0] = np.asarray(sin, np.float32).reshape(-1)
    sm[:, 1] = np.asarray(q_gamma, np.float32).reshape(-1) * SCALE
    sm[:, 2:] = (np.asarray(mask, np.float32).reshape(B, NCH, 128)
                 .transpose(2, 0, 1).reshape(128, B * NCH))
    kv_k = np.asarray(kv_k, np.float32)
    kv_v = np.asarray(kv_v, np.float32)
    Wq = np.asarray(Wq, np.float32)
    Wo = np.asarray(Wo, np.float32)

    in_maps = []
    for c in range(NCORES):
        hs = c * HC
        # packed per (b,h): [kT (128,S) | vT pre-tiled (128,S)]
        kvp = np.empty((B, HC, 128, 2 * S), kv_np)
        kvp[..., :S] = kv_k[:, hs:hs + HC].transpose(0, 1, 3, 2)
        kvp[..., S:] = (kv_v[:, hs:hs + HC].reshape(B, HC, D, NCH, 128)
                        .transpose(0, 1, 4, 3, 2).reshape(B, HC, 128, S))
        in_maps.append({
            "xTp": xtp,
            "wq": np.ascontiguousarray(Wq[:, hs * D:(hs + HC) * D].astype(kv_np)),
            "wo": np.ascontiguousarray(Wo[hs * D:(hs + HC) * D, :].astype(kv_np)),
            "kv": kvp,
            "sm": sm,
        })
    return in_maps


def kernel(x, kv_k, kv_v, cos, sin, mask, Wq, Wo, q_gamma, _trace=False):
    from concourse.bass_utils import run_bass_kernel_spmd

    nc = _get_nc()
    in_maps = _shard_inputs(x, kv_k, kv_v, cos, sin, mask, Wq, Wo, q_gamma)
    res = run_bass_kernel_spmd(nc, in_maps, list(range(NCORES)), trace=_trace)
    acc = np.zeros((B, HID), np.float64)
    for c in range(NCORES):
        acc += res.results[c]["out"].astype(np.float64)
    out = acc.astype(np.float32).reshape(B, 1, HID)
    if _trace:
        return out, res
    return out
